# revision 4
# baseline (speedup 1.0000x reference)
"""Trainium2 Bass kernel for nn_DiffusionModel (100-step linear diffusion sampler).

Math: the reference step is
    x_{t+1} = x_t + beta*(x_t @ W1 + t*s + b) + sqrt(2*beta)*n_t
where W1 = W[:512], s = W[512:].sum(0) (the time-embedding half of the concat
collapses to t*s since the embedding is constant per row). With
A = I + beta*W1 and u_t = sqrt(2beta)*n_t + beta*(t*s + b):
    x_100 = x_0 @ A^100 + sum_t u_t @ A^(99-t)
The seeded JAX noise cannot be reproduced on-device, so n_t is generated on
host (bit-exact threefry) and streamed; all linear algebra runs on device as
one giant PE accumulation: out = sum_p T_p @ A^p  (T_p = u_{99-p}, T_100 = x_0,
padded with zeros to 104 terms). A^p are host-precomputed in float64.

Device layout (per core, batch shard 4096 rows):
  out tiles [m 0..31][128 batch, 512 feat] accumulate in PSUM over p-blocks of
  8 powers; lhsT = 128x128 blocks of T_p^T (fp32r/tf32, pre-rounded on host),
  rhs = A^p rows [128,512] (fp32r). PSUM holds one m-group (8 banks); partial
  sums accumulate into SBUF X_acc across blocks via DVE adds.
"""

import numpy as np

import concourse.bass as bass
import concourse.mybir as mybir
import concourse.tile as tile
from concourse.bass_utils import run_bass_kernel_spmd

F32 = mybir.dt.float32
F32R = mybir.dt.float32r

NUM_STEPS = 100
BETA = 0.01
BATCH = 32768
DIM = 512
N_CORES = 8

# ---------------------------------------------------------------------------
# walrus on this stack supports only ONE sync wait / update per instruction;
# split excess onto same-engine NoOps (engine streams run in block order and
# sequencer NoOps are completion-ordered after prior engine instructions).
_ws_ctr = [0]


def _ws_nop(engine, waits=(), updates=()):
    _ws_ctr[0] += 1
    return mybir.InstNoOp(
        name=f"ws_nop_{_ws_ctr[0]}",
        engine=engine,
        ins=[],
        outs=[],
        sync_info=mybir.SyncInfo(on_wait=list(waits), on_update=list(updates)),
    )


def _ws_fix_block(blk, wl, ul):
    insts = getattr(blk, "instructions", None)
    if insts is None:
        return
    new = []
    for inst in insts:
        for attr in ("body_bb", "blocks"):
            sub = getattr(inst, attr, None)
            if sub is not None:
                for s in sub if isinstance(sub, (list, tuple)) else [sub]:
                    _ws_fix_block(s, wl, ul)
        si = getattr(inst, "sync_info", None)
        waits = list(si.on_wait) if si is not None and si.on_wait else []
        upds = list(si.on_update) if si is not None and si.on_update else []
        if len(waits) > wl:
            excess, keep = waits[:-wl], waits[-wl:]
            for i in range(0, len(excess), wl):
                new.append(_ws_nop(inst.engine, waits=excess[i : i + wl]))
            si.on_wait = keep
        new.append(inst)
        if len(upds) > ul:
            si.on_update = upds[:ul]
            rest = upds[ul:]
            for i in range(0, len(rest), ul):
                new.append(_ws_nop(inst.engine, updates=rest[i : i + ul]))
    blk.instructions = new


def split_excess_waits(nc, wait_limit=1, upd_limit=1):
    for fn in nc.m.functions:
        for blk in fn.blocks:
            _ws_fix_block(blk, wait_limit, upd_limit)
    return nc


# ---------------------------------------------------------------------------
def round_tf32(a: np.ndarray) -> np.ndarray:
    """fp32 -> tf32 (10 mantissa bits), round-to-nearest-even."""
    b = np.ascontiguousarray(a, dtype=np.float32).view(np.uint32)
    round_bit = (b >> np.uint32(13)) & np.uint32(1)
    r = (b + np.uint32(0x0FFF) + round_bit) & np.uint32(0xFFFFE000)
    return r.view(np.float32)


def build_nc(n_p: int, n_groups: int, block: int = 8):
    """Device program, fully unrolled (static DMA offsets — the dynamic-offset
    path exhausts engine registers on this stack).

    n_p: number of power terms. n_groups: m-groups of 8 output tiles of
    [128, DIM] (batch shard = n_groups*8*128 rows). block: powers per PSUM
    accumulation round."""
    n_m = n_groups * 8
    nc = bass.Bass()
    # noise pack: row-block i = (p*4 + k)*(n_groups*2) + g*2 + h holds the
    # lhsT quad (4 m-subtiles side by side) of T_p^T
    nz = nc.dram_tensor(
        "nz", [n_p * 4 * n_groups * 2 * 128, DIM], F32R, kind="ExternalInput"
    )
    # A pack: row-block i = p*4 + k holds A^p[128k:128(k+1), :]
    apow = nc.dram_tensor("apow", [n_p * 4 * 128, DIM], F32R, kind="ExternalInput")
    out = nc.dram_tensor("out", [n_m, 128, DIM], F32, kind="ExternalOutput")

    blocks = [range(s, min(s + block, n_p)) for s in range(0, n_p, block)]

    with tile.TileContext(nc) as tc:
        with (
            tc.tile_pool(name="apool", bufs=40) as apool,
            tc.tile_pool(name="npool", bufs=16) as npool,
            tc.tile_pool(name="xpool", bufs=1) as xpool,
            tc.tile_pool(name="ppool", bufs=8, space="PSUM") as ppool,
        ):
            xacc = []
            for m in range(n_m):
                xt = xpool.tile([128, DIM], F32, tag=f"xacc{m}", name=f"xacc{m}")
                nc.gpsimd.memset(xt[:], 0.0)
                xacc.append(xt)

            for bi, prange in enumerate(blocks):
                a_tiles = {}
                for j, p in enumerate(prange):
                    for k in range(4):
                        at = apool.tile(
                            [128, DIM], F32R, tag="at", name=f"at{bi}_{j}_{k}"
                        )
                        idx = p * 4 + k
                        eng = nc.sync if (j + k) % 2 == 0 else nc.scalar
                        eng.dma_start(at[:], apow[idx * 128 : (idx + 1) * 128, :])
                        a_tiles[(j, k)] = at
                p_last = len(prange) - 1
                for g in range(n_groups):
                    ps = [
                        ppool.tile([128, DIM], F32, tag="ps", name=f"ps{bi}_{g}_{_i}")
                        for _i in range(8)
                    ]
                    for j, p in enumerate(prange):
                        for k in range(4):
                            for h in range(2):
                                nt = npool.tile(
                                    [128, DIM],
                                    F32R,
                                    tag="nt",
                                    name=f"nt{bi}_{g}_{j}_{k}_{h}",
                                )
                                idx = (p * 4 + k) * (n_groups * 2) + g * 2 + h
                                eng = nc.sync if (j + k + h) % 2 == 0 else nc.scalar
                                eng.dma_start(nt[:], nz[idx * 128 : (idx + 1) * 128, :])
                                for mi in range(4):
                                    nc.tensor.matmul(
                                        ps[h * 4 + mi][:],
                                        nt[:, 128 * mi : 128 * (mi + 1)],
                                        a_tiles[(j, k)][:],
                                        start=(j == 0 and k == 0),
                                        stop=(j == p_last and k == 3),
                                    )
                    for ml in range(8):
                        m = g * 8 + ml
                        nc.vector.tensor_tensor(
                            xacc[m][:], ps[ml][:], xacc[m][:], mybir.AluOpType.add
                        )

            for m in range(n_m):
                nc.sync.dma_start(out[m], xacc[m][:])

    split_excess_waits(nc)
    return nc


def _pack_T(T: np.ndarray, n_groups: int) -> np.ndarray:
    """[rows, 512] -> lhsT pack [4k, n_groups, 2h, 128a, 512] (tf32-rounded)."""
    g = n_groups
    v = T.reshape(g, 2, 4, 128, 4, 128)  # g h mi b k a
    v = v.transpose(4, 0, 1, 5, 2, 3)  # k g h a mi b
    return np.ascontiguousarray(v.reshape(4, g, 2, 128, 512))


def _host_prepare(x0, W, b, n_groups=4, cores=N_CORES):
    """Generate noise (bit-exact jax threefry on CPU), fold constants, build
    per-core packed inputs."""
    import jax

    cpu = jax.devices("cpu")[0]
    n_p = NUM_STEPS + 1
    rows_per_core = n_groups * 8 * 128
    assert rows_per_core * cores == x0.shape[0]

    W64 = W.astype(np.float64)
    A = np.eye(DIM, dtype=np.float64) + BETA * W64[:DIM]
    s = W64[DIM:].sum(axis=0)
    b64 = b.astype(np.float64)
    sqrt_2beta = np.sqrt(2.0 * BETA)

    # A^p pack, tf32-rounded, [n_p*4*128, 512]
    apack = np.zeros((n_p, 4, 128, DIM), dtype=np.float32)
    Ap = np.eye(DIM, dtype=np.float64)
    for p in range(NUM_STEPS + 1):
        apack[p] = round_tf32(Ap.astype(np.float32)).reshape(4, 128, DIM)
        if p < NUM_STEPS:
            Ap = Ap @ A
    apack = apack.reshape(n_p * 4 * 128, DIM)

    # per-core noise packs
    nzpack = [
        np.zeros((n_p, 4, n_groups, 2, 128, DIM), dtype=np.float32)
        for _ in range(cores)
    ]

    import jax.numpy as jnp
    from jax import lax

    def gen_step(t):
        k = jax.random.fold_in(jax.random.key(42), t)
        n = jax.random.normal(k, (BATCH, DIM), jnp.float32)
        c_t = (BETA * (t.astype(jnp.float64) * s + b64)).astype(jnp.float32)
        T = (jnp.float32(sqrt_2beta) * n + c_t[None, :]).astype(jnp.float32)
        # tf32 round
        bb = lax.bitcast_convert_type(T, jnp.uint32)
        rb = (bb >> jnp.uint32(13)) & jnp.uint32(1)
        rr = (bb + jnp.uint32(0x0FFF) + rb) & jnp.uint32(0xFFFFE000)
        T = lax.bitcast_convert_type(rr, jnp.float32)
        v = T.reshape(cores, n_groups, 2, 4, 128, 4, 128)
        v = v.transpose(0, 5, 1, 2, 6, 3, 4)  # c k g h a mi b
        return v.reshape(cores, 4, n_groups, 2, 128, DIM)

    with jax.default_device(cpu):
        gen = jax.jit(gen_step)
        for t in range(NUM_STEPS):
            p = NUM_STEPS - 1 - t
            vt = np.asarray(gen(jnp.uint32(t)))
            for c in range(cores):
                nzpack[c][p] = vt[c]
        # p = NUM_STEPS term: x0 (tf32-rounded)
        x0r = round_tf32(x0)
        for c in range(cores):
            nzpack[c][NUM_STEPS] = _pack_T(
                x0r[c * rows_per_core : (c + 1) * rows_per_core], n_groups
            )
    in_maps = []
    for c in range(cores):
        in_maps.append(
            {
                "nz": nzpack[c].reshape(n_p * 4 * n_groups * 2 * 128, DIM),
                "apow": apack,
            }
        )
    return in_maps


def _gather(results, n_groups=4, cores=N_CORES):
    rows_per_core = n_groups * 8 * 128
    out = np.empty((BATCH, DIM), dtype=np.float32)
    for c in range(cores):
        out[c * rows_per_core : (c + 1) * rows_per_core] = results[c]["out"].reshape(
            rows_per_core, DIM
        )
    return out


_CACHED_NC = None


def _get_nc():
    global _CACHED_NC
    if _CACHED_NC is None:
        _CACHED_NC = build_nc(n_p=NUM_STEPS + 1, n_groups=4)
    return _CACHED_NC


def run(x0, W, b, trace=False):
    nc = _get_nc()
    in_maps = _host_prepare(x0, W, b)
    res = run_bass_kernel_spmd(nc, in_maps, list(range(N_CORES)), trace=trace)
    return _gather(res.results), res


def kernel(x0, W, b):
    out, _ = run(x0, W, b, trace=False)
    return out
